# revision 8
# baseline (speedup 1.0000x reference)
"""SMPL-style LBS deformer on 8 TRN2 NeuronCores, data-parallel over batch.

Outputs (matching the reference): posed_verts (B,V,3), posed_joints (B,J,3),
rel_transforms (B,J,4,4).

Split:
  host  - forward kinematics over the 24-joint tree (tiny, sequential):
          rel_transforms + posed_joints.
  device- linear blend skinning over V=6890 vertices (the heavy part):
          per core, 32 batches:
            T[v,(b,m,n)] = sum_j W^T[j,v] * RT[j,(b,m,n)]   (TensorE, K=24)
            posed[v,(b,m)] = sum_n T[v,(b,m,n)] * vh[v,(b,n)] (VectorE)
          vh is the homogeneous vertex, pre-expanded on host over m so the
          device apply is one tensor_tensor mult + one segmented reduce.
"""

import sys
import numpy as np

for _p in ("/opt/trn_rl_repo",):
    if _p not in sys.path:
        sys.path.insert(0, _p)

import concourse.bass as bass
import concourse.mybir as mybir
from concourse.tile import TileContext
from concourse.bass_utils import run_bass_kernel_spmd

# problem constants (hardcoded per harness contract)
B, J, V = 256, 24, 6890
N_CORES = 8
BL = B // N_CORES          # 32 batches per core
VP = 6912                  # V padded to 54*128
NT = VP // 128             # 54 vertex tiles
FD = BL * 3 * 4            # 384 = (b, m, n) free layout
OD = BL * 3                # 96  = (b, m)

PARENTS = np.array([0, 0, 0, 0, 1, 2, 3, 4, 5, 6, 7, 8, 9, 9, 9, 12,
                    13, 14, 16, 17, 18, 19, 20, 21], dtype=np.int64)

_F32 = mybir.dt.float32
_NC_CACHE = {}


def _build_nc():
    nc = bass.Bass()
    wt = nc.dram_tensor("wt", (J, VP), _F32, kind="ExternalInput")
    rtr = nc.dram_tensor("rtr", (J, FD), _F32, kind="ExternalInput")
    vx = nc.dram_tensor("vx", (VP, FD), _F32, kind="ExternalInput")
    out = nc.dram_tensor("out", (VP, OD), _F32, kind="ExternalOutput")

    # Raw bass with manual semaphores: the bass2jax/walrus path here allows
    # only ONE fused sync-wait per instruction, so extra dependencies are
    # expressed as standalone wait_ge instructions, and each multi-part
    # input is loaded by a single DMA (one completion semaphore each).
    AP = bass.AP
    NCHUNK = 6
    CT = NT // NCHUNK          # 9 tiles per vx chunk
    NPS = 8                    # psum buffers (one bank each)
    import contextlib
    with contextlib.ExitStack() as ctx:
        rtr_sb = ctx.enter_context(nc.sbuf_tensor("rtr_sb", [J, FD], _F32))
        wts_all = ctx.enter_context(nc.sbuf_tensor("wts_all", [J, VP], _F32))
        vx_all = ctx.enter_context(nc.sbuf_tensor("vx_all", [128, NT * FD], _F32))
        ot_all = ctx.enter_context(nc.sbuf_tensor("ot_all", [128, NT * OD], _F32))
        prods = [ctx.enter_context(nc.sbuf_tensor(f"prod{i}", [128, FD], _F32))
                 for i in range(2)]
        pss = [ctx.enter_context(nc.psum_tensor(f"ps{i}", [128, FD], _F32))
               for i in range(NPS)]
        s_dr = ctx.enter_context(nc.semaphore("s_dr"))
        s_dw = ctx.enter_context(nc.semaphore("s_dw"))
        s_dv = [ctx.enter_context(nc.semaphore(f"s_dv{k}"))
                for k in range(NCHUNK)]
        s_pe = ctx.enter_context(nc.semaphore("s_pe"))
        s_tt = ctx.enter_context(nc.semaphore("s_tt"))
        s_rd = ctx.enter_context(nc.semaphore("s_rd"))
        s_do = ctx.enter_context(nc.semaphore("s_do"))
        block = ctx.enter_context(nc.Block())

        @block.sync
        def _(sync):
            sync.dma_start(
                AP(rtr_sb, 0, [[FD, J], [1, FD]]),
                AP(rtr, 0, [[FD, J], [1, FD]]),
            ).then_inc(s_dr, 16)
            sync.dma_start(
                AP(wts_all, 0, [[VP, J], [1, VP]]),
                AP(wt, 0, [[VP, J], [1, VP]]),
            ).then_inc(s_dw, 16)
            for k in range(NCHUNK):
                sync.dma_start(
                    AP(vx_all, k * CT * FD,
                       [[NT * FD, 128], [FD, CT], [1, FD]]),
                    AP(vx, k * CT * 128 * FD,
                       [[FD, 128], [128 * FD, CT], [1, FD]]),
                ).then_inc(s_dv[k], 16)

        @block.tensor
        def _(tensor):
            tensor.wait_ge(s_dr, 16)
            tensor.wait_ge(s_dw, 16)
            for t in range(NT):
                if t >= NPS:
                    tensor.wait_ge(s_tt, t - NPS + 1)
                tensor.matmul(
                    AP(pss[t % NPS], 0, [[FD, 128], [1, FD]]),
                    AP(wts_all, t * 128, [[VP, J], [1, 128]]),
                    AP(rtr_sb, 0, [[FD, J], [1, FD]]),
                    start=True, stop=True,
                ).then_inc(s_pe, 1)

        @block.vector
        def _(vector):
            for t in range(NT):
                if t % CT == 0:
                    vector.wait_ge(s_dv[t // CT], 16)
                vector.wait_ge(s_pe, t + 1)
                pr = prods[t % 2]
                vector.tensor_tensor(
                    out=AP(pr, 0, [[FD, 128], [1, FD]]),
                    in0=AP(pss[t % NPS], 0, [[FD, 128], [1, FD]]),
                    in1=AP(vx_all, t * FD, [[NT * FD, 128], [1, FD]]),
                    op=mybir.AluOpType.mult,
                ).then_inc(s_tt, 1)
                vector.tensor_reduce(
                    out=AP(ot_all, t * OD, [[NT * OD, 128], [1, OD]]),
                    in_=AP(pr, 0, [[FD, 128], [4, OD], [1, 4]]),
                    axis=mybir.AxisListType.X,
                    op=mybir.AluOpType.add,
                ).then_inc(s_rd, 1)

        @block.gpsimd
        def _(gpsimd):
            for t in range(NT):
                gpsimd.wait_ge(s_rd, t + 1)
                gpsimd.dma_start(
                    AP(out, t * 128 * OD, [[OD, 128], [1, OD]]),
                    AP(ot_all, t * OD, [[NT * OD, 128], [1, OD]]),
                ).then_inc(s_do, 16)
            gpsimd.wait_ge(s_do, 16 * NT)
    return nc


def _forward_kinematics(rot_mats, joints):
    """Steps 1-5 of the reference in float64; returns (posed_joints, rel_T)."""
    r = rot_mats.astype(np.float64)
    j = joints.astype(np.float64)
    rel = j.copy()
    rel[:, 1:] -= j[:, PARENTS[1:]]
    tl = np.zeros((B, J, 4, 4), dtype=np.float64)
    tl[:, :, :3, :3] = r
    tl[:, :, :3, 3] = rel
    tl[:, :, 3, 3] = 1.0
    a = np.empty_like(tl)
    a[:, 0] = tl[:, 0]
    for i in range(1, J):
        a[:, i] = a[:, PARENTS[i]] @ tl[:, i]
    posed_joints = a[:, :, :3, 3]
    tj = np.einsum('bjmn,bjn->bjm', a[:, :, :, :3], j)
    rel_t = a.copy()
    rel_t[:, :, :, 3] -= tj
    return posed_joints, rel_t


def kernel(rot_mats, joints, vertices, lbs_weights):
    posed_joints64, rel_t64 = _forward_kinematics(rot_mats, joints)
    posed_joints = posed_joints64.astype(np.float32)
    rel_transforms = rel_t64.astype(np.float32)

    # replicated blend weights, transposed + padded: (J, VP)
    wt = np.zeros((J, VP), dtype=np.float32)
    wt[:, :V] = np.ascontiguousarray(lbs_weights.T.astype(np.float32))

    in_maps = []
    for c in range(N_CORES):
        bs = slice(c * BL, (c + 1) * BL)
        # RT rearranged (j, b, m, n), rows m=0..2 only
        rtr = np.ascontiguousarray(
            rel_transforms[bs][:, :, 0:3, :].transpose(1, 0, 2, 3)
        ).reshape(J, FD)
        # homogeneous verts (v, b, n) broadcast over m -> (VP, b, m, n)
        vh = np.ones((BL, V, 4), dtype=np.float32)
        vh[:, :, :3] = vertices[bs]
        vx = np.zeros((VP, FD), dtype=np.float32)
        vx[:V] = np.broadcast_to(
            vh.transpose(1, 0, 2)[:, :, None, :], (V, BL, 3, 4)
        ).reshape(V, FD)
        in_maps.append({"wt": wt, "rtr": rtr, "vx": vx})

    if "nc" not in _NC_CACHE:
        _NC_CACHE["nc"] = _build_nc()
    res = run_bass_kernel_spmd(_NC_CACHE["nc"], in_maps,
                               core_ids=list(range(N_CORES)))

    posed_verts = np.empty((B, V, 3), dtype=np.float32)
    for c in range(N_CORES):
        o = np.asarray(res.results[c]["out"])[:V]          # (V, BL*3)
        posed_verts[c * BL:(c + 1) * BL] = o.reshape(V, BL, 3).transpose(1, 0, 2)

    return posed_verts, posed_joints, rel_transforms
